# revision 9
# baseline (speedup 1.0000x reference)
"""Trainium2 Bass kernel: document-level LSTM (B=64, T=1024, D=300, H=512)
with mean-over-time pooling and a sigmoid dense head.

Strategy (8 NeuronCores, data-parallel over batch, B=8 per core):

  Everything on-chip is gate-major ("transposed"): gate tensors live as
  [128 partitions = position-within-128-chunk, free = (chunk, batch)].
  The LSTM state h is kept as h.T tiles [128, (k-chunk, batch)], which is
  exactly the moving operand the recurrence matmul needs, so the kernel
  contains no transposes at all.

  Per time step, gates.T[m] = sum_k Wh[k,m].T @ h.T[k]: the fixed Wh tiles
  [128,128] (fp16, FastWeightLoad) are the stationary operand, h.T [128,8]
  slices are the moving operand.  The input projection x@Wx+b is computed
  in 64-step blocks fused into the loop (never leaves SBUF), in the same
  gate-major layout, and added during the elementwise phase.  Gate order
  along the 16 m-chunks is [f, i, j, o] so the c-update chain starts early
  and only sigmoid(o)*tanh(c_new) sits on the per-step critical tail.

  The elementwise phase runs on ScalarE (sigmoid/tanh) and VectorE
  (adds/muls) on [128, 32] tiles and hides under the PE weight-load shadow.
  Each gate gets its own PSUM bank so VectorE reads of one gate never
  serialize against PE writes of the next (bank-granular overlap tracking).
"""
import sys
sys.path.insert(0, "/opt/trn_rl_repo")
import numpy as np

B = 8          # batch per core
BF = 64        # full batch
H = 512
G4 = 2048
D = 300
D_PAD = 384
T = 1024
BS = 64        # time block size
KC = 4         # H / 128
MC = 16        # 4H / 128
N_CORES = 8
WS = 128.0     # fp8 weight scale: Wh, ident stored as WS*value in fp8e4

_CACHE = {}


def _build():
    import concourse.mybir as mybir
    import concourse.tile as tile
    from concourse import bacc

    F32 = mybir.dt.float32
    F16 = mybir.dt.float16
    F8 = mybir.dt.float8e4
    AF = mybir.ActivationFunctionType
    OP = mybir.AluOpType
    NB = T // BS

    nc = bacc.Bacc("TRN2", target_bir_lowering=False, debug=False,
                   num_devices=N_CORES)

    ex_d = nc.dram_tensor("ex", [B, T, D_PAD], F16, kind="ExternalInput")
    ident_d = nc.dram_tensor("ident", [128, 128], F8, kind="ExternalInput")
    wh_d = nc.dram_tensor("wh", [128, KC * MC * 128], F8, kind="ExternalInput")
    wx_d = nc.dram_tensor("wx", [128, 3 * MC * 128], F16, kind="ExternalInput")
    bias_d = nc.dram_tensor("bias", [128, MC], F32, kind="ExternalInput")
    wd_d = nc.dram_tensor("wd", [128, KC], F32, kind="ExternalInput")
    bd_d = nc.dram_tensor("bd", [B, 1], F32, kind="ExternalInput")
    out_d = nc.dram_tensor("out", [B, 1], F32, kind="ExternalOutput")

    with tile.TileContext(nc) as tc:
        with (
            tc.tile_pool(name="w", bufs=1) as wpool,
            tc.tile_pool(name="xp", bufs=2) as xppool,
            tc.tile_pool(name="ex", bufs=2) as expool,
            tc.tile_pool(name="ew", bufs=4) as ewpool,
            tc.tile_pool(name="st", bufs=4) as stpool,
            tc.tile_pool(name="p1", bufs=2, space="PSUM") as p1pool,
            tc.tile_pool(name="pg", bufs=1, space="PSUM") as gpool,
            tc.tile_pool(name="pd", bufs=1, space="PSUM") as pdpool,
        ):
            wh = wpool.tile([128, KC * MC * 128], F8)
            wx = wpool.tile([128, 3 * MC * 128], F16)
            bias = wpool.tile([128, MC], F32)
            wd = wpool.tile([128, KC], F32)
            bd = wpool.tile([B, 1], F32)
            nc.sync.dma_start(out=wh[:], in_=wh_d[:])
            nc.sync.dma_start(out=wx[:], in_=wx_d[:])
            nc.sync.dma_start(out=bias[:], in_=bias_d[:])
            nc.sync.dma_start(out=wd[:], in_=wd_d[:])
            nc.sync.dma_start(out=bd[:], in_=bd_d[:])
            ident = wpool.tile([128, 128], F8, tag="ident", name="ident")
            nc.sync.dma_start(out=ident[:], in_=ident_d[:])

            h = stpool.tile([128, KC * B], F16, tag="h")
            c = stpool.tile([128, KC * B], F32, tag="c")
            acc = stpool.tile([128, KC * B], F32, tag="acc")
            nc.vector.memset(h[:], 0.0)
            nc.vector.memset(c[:], 0.0)
            nc.vector.memset(acc[:], 0.0)

            def load_ex(bb):
                t0 = bb * BS
                tiles = []
                for k in range(3):
                    et = expool.tile([128, BS * B], F16, tag=f"ex{k}",
                                     name=f"ex{k}")
                    etv = et[:].rearrange("p (t b) -> p t b", t=BS, b=B)
                    for bi in range(B):
                        src = ex_d[bi, t0:t0 + BS, k * 128:(k + 1) * 128]
                        nc.sync.dma_start(out=etv[:, :, bi],
                                          in_=src.rearrange("t d -> d t"))
                    tiles.append(et)
                return tiles

            def phase1_mgroup(xp_t, ex_tiles, m):
                ps = p1pool.tile([128, BS * B], F32, tag="p1", name="p1")
                for k in range(3):
                    nc.tensor.matmul(
                        ps[:],
                        wx[:, (k * MC + m) * 128:(k * MC + m + 1) * 128],
                        ex_tiles[k][:],
                        start=(k == 0), stop=(k == 2),
                    )
                xv = xp_t[:].rearrange("p (t m b) -> p t m b", t=BS, m=MC, b=B)
                psv = ps[:].rearrange("p (t b) -> p t b", t=BS, b=B)
                # split into 4 pieces so no single ACT op can block the
                # per-step sigmoid/tanh chain for long (ACT queue is in-order)
                for q in range(4):
                    nc.scalar.activation(
                        out=xv[:, q * 16:(q + 1) * 16, m, :],
                        in_=psv[:, q * 16:(q + 1) * 16, :],
                        func=AF.Identity, bias=bias[:, m:m + 1], scale=1.0,
                    )

            ex_tiles = load_ex(0)
            xp_cur = xppool.tile([128, BS * MC * B], F16, tag="xp", name="xp")
            for m in range(MC):
                phase1_mgroup(xp_cur, ex_tiles, m)
            xp_next = None

            for t in range(T):
                bb, tloc = divmod(t, BS)
                if tloc == 0 and bb + 1 < NB:
                    ex_tiles = load_ex(bb + 1)
                    xp_next = xppool.tile([128, BS * MC * B], F16, tag="xp",
                                          name="xp")
                if tloc % 4 == 0 and bb + 1 < NB:
                    phase1_mgroup(xp_next, ex_tiles, tloc // 4)

                sig = {}
                cf = u = c_new = tanh_c = None
                # seed all four gates' accumulations with xp on the PE itself
                # (ps = I.T @ xp_slice) BEFORE any h-dependent matmul: the PE
                # queue is in-order, so emitting the seeds first lets all four
                # execute during the previous step's tail stall, and the
                # activations below read PSUM directly.
                ps_g = []
                for g in range(4):
                    ps = gpool.tile([128, 4 * B], F32, tag=f"pg{g}",
                                    name=f"pg{g}")
                    xp_slice = xp_cur[:, tloc * MC * B + g * 4 * B:
                                      tloc * MC * B + (g + 1) * 4 * B]
                    nc.tensor.matmul(ps[:], ident[:], xp_slice,
                                     start=True, stop=False,
                                     skip_group_check=True)
                    ps_g.append(ps)
                for g in range(4):  # gate order: f, i, j, o
                    ps = ps_g[g]
                    for mm in range(4):
                        m = g * 4 + mm
                        for k in range(KC):
                            nc.tensor.matmul(
                                ps[:, mm * B:(mm + 1) * B],
                                wh[:, (k * MC + m) * 128:(k * MC + m + 1) * 128],
                                h[:, k * B:(k + 1) * B],
                                start=False, stop=(k == KC - 1),
                                skip_group_check=True,
                            )
                    st = ewpool.tile([128, 4 * B], F32, tag=f"s{g}",
                                     name=f"s{g}")
                    nc.scalar.activation(out=st[:], in_=ps[:],
                                         func=AF.Tanh if g == 2 else AF.Sigmoid,
                                         scale=1.0 / WS)
                    sig[g] = st
                    if g == 0:
                        # on GPSIMD (otherwise idle; SBUF-only operands) so it
                        # runs concurrently with the DVE's u = sig(i)*tanh(j)
                        cf = ewpool.tile([128, 4 * B], F32, tag="cf", name="cf")
                        nc.gpsimd.tensor_tensor(cf[:], c[:], st[:], OP.mult)
                    elif g == 2:
                        u = ewpool.tile([128, 4 * B], F32, tag="u", name="u")
                        nc.vector.tensor_tensor(u[:], sig[1][:], st[:], OP.mult)
                        c_new = stpool.tile([128, KC * B], F32, tag="c",
                                            name="c")
                        nc.vector.tensor_tensor(c_new[:], cf[:], u[:], OP.add)
                        tanh_c = ewpool.tile([128, 4 * B], F32, tag="tc",
                                             name="tc")
                        nc.scalar.activation(out=tanh_c[:], in_=c_new[:],
                                             func=AF.Tanh)
                h_new = stpool.tile([128, KC * B], F16, tag="h", name="h")
                nc.vector.tensor_tensor(h_new[:], tanh_c[:], sig[3][:], OP.mult)
                acc_new = stpool.tile([128, KC * B], F32, tag="acc", name="acc")
                nc.gpsimd.tensor_tensor(acc_new[:], acc[:], h_new[:], OP.add)
                h, c, acc = h_new, c_new, acc_new

                if tloc == BS - 1 and bb + 1 < NB:
                    xp_cur = xp_next

            pd = pdpool.tile([B, 1], F32, tag="pd")
            for k in range(KC):
                nc.tensor.matmul(pd[:], acc[:, k * B:(k + 1) * B],
                                 wd[:, k:k + 1],
                                 start=(k == 0), stop=(k == KC - 1))
            res = ewpool.tile([B, 1], F32, tag="res")
            nc.scalar.activation(out=res[:], in_=pd[:], func=AF.Sigmoid,
                                 bias=bd[:], scale=1.0 / T)
            nc.sync.dma_start(out=out_d[:], in_=res[:])

    nc.compile()
    return nc


def _get_exec():
    if "exec" in _CACHE:
        return _CACHE["exec"]
    import jax
    import concourse.mybir as mybir
    from concourse import bass2jax
    from jax.sharding import Mesh, PartitionSpec, NamedSharding
    from jax.experimental.shard_map import shard_map

    nc = _build()
    bass2jax.install_neuronx_cc_hook()
    partition_name = (nc.partition_id_tensor.name
                      if nc.partition_id_tensor else None)
    in_names, out_names, out_avals = [], [], []
    for alloc in nc.m.functions[0].allocations:
        if not isinstance(alloc, mybir.MemoryLocationSet):
            continue
        name = alloc.memorylocations[0].name
        if alloc.kind == "ExternalInput":
            if name != partition_name:
                in_names.append(name)
        elif alloc.kind == "ExternalOutput":
            out_names.append(name)
            out_avals.append(jax.core.ShapedArray(
                tuple(alloc.tensor_shape), mybir.dt.np(alloc.dtype)))
    n_params = len(in_names)
    all_in = in_names + out_names + ([partition_name] if partition_name else [])

    def _body(*a):
        operands = list(a)
        if partition_name is not None:
            operands.append(bass2jax.partition_id_tensor())
        return tuple(bass2jax._bass_exec_p.bind(
            *operands, out_avals=tuple(out_avals), in_names=tuple(all_in),
            out_names=tuple(out_names), lowering_input_output_aliases=(),
            sim_require_finite=True, sim_require_nnan=True, nc=nc))

    devices = jax.devices()[:N_CORES]
    mesh = Mesh(np.asarray(devices), ("core",))
    jitted = jax.jit(
        shard_map(_body, mesh=mesh,
                  in_specs=(PartitionSpec("core"),) * (n_params + len(out_avals)),
                  out_specs=(PartitionSpec("core"),) * len(out_names),
                  check_rep=False),
        keep_unused=True)
    shard = NamedSharding(mesh, PartitionSpec("core"))
    state = (jitted, in_names, out_avals, mesh, shard)
    _CACHE["exec"] = state
    return state


def _prep_in_maps(essays, W_lstm, b_lstm, W_dense, b_dense):
    perm = np.concatenate([
        np.arange(1024, 1536),   # f
        np.arange(0, 512),       # i
        np.arange(512, 1024),    # j
        np.arange(1536, 2048),   # o
    ])
    Wx = W_lstm[:D][:, perm]
    Wh = W_lstm[D:][:, perm]
    b_eff = b_lstm[perm].astype(np.float32).copy()
    b_eff[0:512] += 1.0  # TF BasicLSTMCell forget bias ([f] block is first)

    Wx_pad = np.zeros((D_PAD, G4), np.float32)
    Wx_pad[:D] = Wx
    wx_packed = Wx_pad.reshape(3, 128, MC, 128).transpose(1, 0, 2, 3) \
        .reshape(128, 3 * MC * 128).astype(np.float16)
    import ml_dtypes
    wh_packed = (Wh * WS).reshape(KC, 128, MC, 128).transpose(1, 0, 2, 3) \
        .reshape(128, KC * MC * 128).astype(ml_dtypes.float8_e4m3)
    bias_t = b_eff.reshape(MC, 128).T.copy().astype(np.float32)
    wd_t = W_dense[:, 0].reshape(KC, 128).T.copy().astype(np.float32)
    bd_t = np.full((B, 1), float(b_dense[0]), np.float32)

    ex_pad = np.zeros((BF, T, D_PAD), np.float16)
    ex_pad[:, :, :D] = essays.astype(np.float16)

    return [{
        "ex": ex_pad[core * B:(core + 1) * B],
        "wh": wh_packed,
        "wx": wx_packed,
        "bias": bias_t,
        "wd": wd_t,
        "bd": bd_t,
        "ident": (np.eye(128, dtype=np.float32) * WS).astype(
            ml_dtypes.float8_e4m3),
    } for core in range(N_CORES)]


def kernel(essays, W_lstm, b_lstm, W_dense, b_dense):
    import jax
    essays = np.asarray(essays, np.float32)
    W_lstm = np.asarray(W_lstm, np.float32)
    b_lstm = np.asarray(b_lstm, np.float32)
    W_dense = np.asarray(W_dense, np.float32)
    b_dense = np.asarray(b_dense, np.float32)

    jitted, in_names, out_avals, mesh, shard = _get_exec()
    in_maps = _prep_in_maps(essays, W_lstm, b_lstm, W_dense, b_dense)
    concat_in = [np.concatenate([in_maps[c][nm] for c in range(N_CORES)],
                                axis=0) for nm in in_names]
    concat_zeros = [np.zeros((N_CORES * a.shape[0], *a.shape[1:]), a.dtype)
                    for a in out_avals]
    dev_in = [jax.device_put(a, shard) for a in concat_in]
    dev_zeros = [jax.device_put(a, shard) for a in concat_zeros]
    out = jitted(*dev_in, *dev_zeros)
    jax.block_until_ready(out)
    preds = np.asarray(out[0]).reshape(-1).astype(np.float32)
    return preds


# expose the device-resident runner for timing harnesses
def _timed_run(essays, W_lstm, b_lstm, W_dense, b_dense, n_launch=9,
               trials=4):
    """Return (preds, per_launch_seconds_median) using pipelined launches."""
    import time, jax
    jitted, in_names, out_avals, mesh, shard = _get_exec()
    in_maps = _prep_in_maps(np.asarray(essays, np.float32),
                            np.asarray(W_lstm, np.float32),
                            np.asarray(b_lstm, np.float32),
                            np.asarray(W_dense, np.float32),
                            np.asarray(b_dense, np.float32))
    concat_in = [np.concatenate([in_maps[c][nm] for c in range(N_CORES)],
                                axis=0) for nm in in_names]
    concat_zeros = [np.zeros((N_CORES * a.shape[0], *a.shape[1:]), a.dtype)
                    for a in out_avals]
    dev_in = [jax.device_put(a, shard) for a in concat_in]
    dev_zeros = [jax.device_put(a, shard) for a in concat_zeros]

    out = jitted(*dev_in, *dev_zeros)
    jax.block_until_ready(out)
    preds = np.asarray(out[0]).reshape(-1).astype(np.float32)

    def timed(K):
        t0 = time.perf_counter()
        o = None
        for _ in range(K):
            o = jitted(*dev_in, *dev_zeros)
        jax.block_until_ready(o)
        return time.perf_counter() - t0

    # pipelined-launch slope: marginal cost of 16 extra launches.  This is
    # an upper bound on device time (host dispatch overlaps device exec).
    timed(2)  # warm
    margins = []
    for _ in range(trials):
        t3 = timed(3)
        t19 = timed(19)
        margins.append((t19 - t3) / 16)
    return preds, float(np.median(margins))



# revision 12
# speedup vs baseline: 1.6656x; 1.6656x over previous
"""Trainium2 Bass kernel: document-level LSTM (B=64, T=1024, D=300, H=512)
with mean-over-time pooling and a sigmoid dense head.

Strategy (8 NeuronCores, data-parallel over batch, B=8 per core):

  Everything on-chip is gate-major ("transposed"): gate tensors live as
  [128 partitions = position-within-128-chunk, free = (chunk, batch)].
  The LSTM state h is kept as h.T tiles [128, (k-chunk, batch)], which is
  exactly the moving operand the recurrence matmul needs, so the kernel
  contains no transposes at all.

  Per time step, gates.T[m] = sum_k Wh[k,m].T @ h.T[k]: the fixed Wh tiles
  [128,128] (fp16, FastWeightLoad) are the stationary operand, h.T [128,8]
  slices are the moving operand.  The input projection x@Wx+b is computed
  in 64-step blocks fused into the loop (never leaves SBUF), in the same
  gate-major layout, and added during the elementwise phase.  Gate order
  along the 16 m-chunks is [f, i, j, o] so the c-update chain starts early
  and only sigmoid(o)*tanh(c_new) sits on the per-step critical tail.

  The elementwise phase runs on ScalarE (sigmoid/tanh) and VectorE
  (adds/muls) on [128, 32] tiles and hides under the PE weight-load shadow.
  Each gate gets its own PSUM bank so VectorE reads of one gate never
  serialize against PE writes of the next (bank-granular overlap tracking).
"""
import sys
sys.path.insert(0, "/opt/trn_rl_repo")
import numpy as np

B = 8          # batch per core
BF = 64        # full batch
H = 512
G4 = 2048
D = 300
D_PAD = 384
T = 1024
BS = 64        # time block size
KC = 4         # H / 128
MC = 16        # 4H / 128
N_CORES = 8
WS = 128.0     # fp8 weight scale: Wh, ident stored as WS*value in fp8e4

_CACHE = {}


def _build():
    import concourse.mybir as mybir
    import concourse.tile as tile
    from concourse import bacc

    F32 = mybir.dt.float32
    F16 = mybir.dt.float16
    F8 = mybir.dt.float8e4
    AF = mybir.ActivationFunctionType
    OP = mybir.AluOpType
    NB = T // BS

    nc = bacc.Bacc("TRN2", target_bir_lowering=False, debug=False,
                   num_devices=N_CORES)

    ex_d = nc.dram_tensor("ex", [B, T, D_PAD], F16, kind="ExternalInput")
    ident_d = nc.dram_tensor("ident", [128, 128], F8, kind="ExternalInput")
    wh_d = nc.dram_tensor("wh", [128, KC * MC * 128], F8, kind="ExternalInput")
    wx_d = nc.dram_tensor("wx", [128, 3 * MC * 128], F16, kind="ExternalInput")
    bias_d = nc.dram_tensor("bias", [128, MC], F32, kind="ExternalInput")
    wd_d = nc.dram_tensor("wd", [128, KC], F32, kind="ExternalInput")
    bd_d = nc.dram_tensor("bd", [B, 1], F32, kind="ExternalInput")
    out_d = nc.dram_tensor("out", [B, 1], F32, kind="ExternalOutput")

    with tile.TileContext(nc) as tc:
        with (
            tc.tile_pool(name="w", bufs=1) as wpool,
            tc.tile_pool(name="xp", bufs=2) as xppool,
            tc.tile_pool(name="ex", bufs=2) as expool,
            tc.tile_pool(name="ew", bufs=4) as ewpool,
            tc.tile_pool(name="st", bufs=4) as stpool,
            tc.tile_pool(name="p1", bufs=2, space="PSUM") as p1pool,
            tc.tile_pool(name="pg", bufs=1, space="PSUM") as gpool,
            tc.tile_pool(name="pd", bufs=1, space="PSUM") as pdpool,
        ):
            wh = wpool.tile([128, KC * MC * 128], F8)
            wx = wpool.tile([128, 3 * MC * 128], F16)
            bias = wpool.tile([128, MC], F32)
            wd = wpool.tile([128, KC], F32)
            bd = wpool.tile([B, 1], F32)
            nc.sync.dma_start(out=wh[:], in_=wh_d[:])
            nc.sync.dma_start(out=wx[:], in_=wx_d[:])
            nc.sync.dma_start(out=bias[:], in_=bias_d[:])
            nc.sync.dma_start(out=wd[:], in_=wd_d[:])
            nc.sync.dma_start(out=bd[:], in_=bd_d[:])
            ident = wpool.tile([128, 128], F8, tag="ident", name="ident")
            nc.sync.dma_start(out=ident[:], in_=ident_d[:])

            h = stpool.tile([128, KC * B], F16, tag="h")
            c = stpool.tile([128, KC * B], F32, tag="c")
            acc = stpool.tile([128, KC * B], F32, tag="acc")
            nc.vector.memset(h[:], 0.0)
            nc.vector.memset(c[:], 0.0)
            nc.vector.memset(acc[:], 0.0)

            def load_ex(bb):
                t0 = bb * BS
                tiles = []
                for k in range(3):
                    et = expool.tile([128, BS * B], F16, tag=f"ex{k}",
                                     name=f"ex{k}")
                    etv = et[:].rearrange("p (t b) -> p t b", t=BS, b=B)
                    for bi in range(B):
                        src = ex_d[bi, t0:t0 + BS, k * 128:(k + 1) * 128]
                        nc.sync.dma_start(out=etv[:, :, bi],
                                          in_=src.rearrange("t d -> d t"))
                    tiles.append(et)
                return tiles

            p1_carry = {}

            def phase1_mm(xp_t, ex_tiles, idx):
                # one matmul per step (idx in [0, 48)): m-group idx//3,
                # k-chunk idx%3.  The bias-add runs on DVE (not ACT) so the
                # per-step sigmoid/tanh chain never queues behind it.
                m, k = divmod(idx, 3)
                if k == 0:
                    p1_carry["ps"] = p1pool.tile([128, BS * B], F32, tag="p1",
                                                 name="p1")
                ps = p1_carry["ps"]
                nc.tensor.matmul(
                    ps[:],
                    wx[:, (k * MC + m) * 128:(k * MC + m + 1) * 128],
                    ex_tiles[k][:],
                    start=(k == 0), stop=(k == 2),
                )
                if k == 2:
                    xv = xp_t[:].rearrange("p (t m b) -> p t m b",
                                           t=BS, m=MC, b=B)
                    psv = ps[:].rearrange("p (t b) -> p t b", t=BS, b=B)
                    nc.vector.tensor_scalar_add(
                        out=xv[:, :, m, :], in0=psv[:, :, :],
                        scalar1=bias[:, m:m + 1])

            ex_tiles = load_ex(0)
            xp_cur = xppool.tile([128, BS * MC * B], F16, tag="xp", name="xp")
            for idx in range(3 * MC):
                phase1_mm(xp_cur, ex_tiles, idx)
            xp_next = None

            for t in range(T):
                bb, tloc = divmod(t, BS)
                if tloc == 0 and bb + 1 < NB:
                    ex_tiles = load_ex(bb + 1)
                    xp_next = xppool.tile([128, BS * MC * B], F16, tag="xp",
                                          name="xp")
                if tloc % 4 != 3 and bb + 1 < NB:
                    phase1_mm(xp_next, ex_tiles, tloc - tloc // 4)

                sig = {}
                cf = u = c_new = tanh_c = None
                # seed all four gates' accumulations with xp on the PE itself
                # (ps = I.T @ xp_slice) BEFORE any h-dependent matmul: the PE
                # queue is in-order, so emitting the seeds first lets all four
                # execute during the previous step's tail stall, and the
                # activations below read PSUM directly.
                ps_g = []
                for g in range(4):
                    ps = gpool.tile([128, 4 * B], F32, tag=f"pg{g}",
                                    name=f"pg{g}")
                    xp_slice = xp_cur[:, tloc * MC * B + g * 4 * B:
                                      tloc * MC * B + (g + 1) * 4 * B]
                    nc.tensor.matmul(ps[:], ident[:], xp_slice,
                                     start=True, stop=False,
                                     skip_group_check=True)
                    ps_g.append(ps)
                for g in range(4):  # gate order: f, i, j, o
                    ps = ps_g[g]
                    for mm in range(4):
                        m = g * 4 + mm
                        for k in range(KC):
                            nc.tensor.matmul(
                                ps[:, mm * B:(mm + 1) * B],
                                wh[:, (k * MC + m) * 128:(k * MC + m + 1) * 128],
                                h[:, k * B:(k + 1) * B],
                                start=False, stop=(k == KC - 1),
                                skip_group_check=True,
                            )
                    st = ewpool.tile([128, 4 * B], F32, tag=f"s{g}",
                                     name=f"s{g}")
                    nc.scalar.activation(out=st[:], in_=ps[:],
                                         func=AF.Tanh if g == 2 else AF.Sigmoid,
                                         scale=1.0 / WS)
                    sig[g] = st
                    if g == 0:
                        # on GPSIMD (otherwise idle; SBUF-only operands) so it
                        # runs concurrently with the DVE's u = sig(i)*tanh(j)
                        cf = ewpool.tile([128, 4 * B], F32, tag="cf", name="cf")
                        nc.gpsimd.tensor_tensor(cf[:], c[:], st[:], OP.mult)
                    elif g == 2:
                        u = ewpool.tile([128, 4 * B], F32, tag="u", name="u")
                        nc.vector.tensor_tensor(u[:], sig[1][:], st[:], OP.mult)
                        c_new = stpool.tile([128, KC * B], F32, tag="c",
                                            name="c")
                        nc.vector.tensor_tensor(c_new[:], cf[:], u[:], OP.add)
                        tanh_c = ewpool.tile([128, 4 * B], F32, tag="tc",
                                             name="tc")
                        nc.scalar.activation(out=tanh_c[:], in_=c_new[:],
                                             func=AF.Tanh)
                h_new = stpool.tile([128, KC * B], F16, tag="h", name="h")
                nc.vector.tensor_tensor(h_new[:], tanh_c[:], sig[3][:], OP.mult)
                acc_new = stpool.tile([128, KC * B], F32, tag="acc", name="acc")
                nc.gpsimd.tensor_tensor(acc_new[:], acc[:], h_new[:], OP.add)
                h, c, acc = h_new, c_new, acc_new

                if tloc == BS - 1 and bb + 1 < NB:
                    xp_cur = xp_next

            pd = pdpool.tile([B, 1], F32, tag="pd")
            for k in range(KC):
                nc.tensor.matmul(pd[:], acc[:, k * B:(k + 1) * B],
                                 wd[:, k:k + 1],
                                 start=(k == 0), stop=(k == KC - 1))
            res = ewpool.tile([B, 1], F32, tag="res")
            nc.scalar.activation(out=res[:], in_=pd[:], func=AF.Sigmoid,
                                 bias=bd[:], scale=1.0 / T)
            nc.sync.dma_start(out=out_d[:], in_=res[:])

    nc.compile()
    return nc


def _get_exec():
    if "exec" in _CACHE:
        return _CACHE["exec"]
    import jax
    import concourse.mybir as mybir
    from concourse import bass2jax
    from jax.sharding import Mesh, PartitionSpec, NamedSharding
    from jax.experimental.shard_map import shard_map

    nc = _build()
    bass2jax.install_neuronx_cc_hook()
    partition_name = (nc.partition_id_tensor.name
                      if nc.partition_id_tensor else None)
    in_names, out_names, out_avals = [], [], []
    for alloc in nc.m.functions[0].allocations:
        if not isinstance(alloc, mybir.MemoryLocationSet):
            continue
        name = alloc.memorylocations[0].name
        if alloc.kind == "ExternalInput":
            if name != partition_name:
                in_names.append(name)
        elif alloc.kind == "ExternalOutput":
            out_names.append(name)
            out_avals.append(jax.core.ShapedArray(
                tuple(alloc.tensor_shape), mybir.dt.np(alloc.dtype)))
    n_params = len(in_names)
    all_in = in_names + out_names + ([partition_name] if partition_name else [])

    def _body(*a):
        operands = list(a)
        if partition_name is not None:
            operands.append(bass2jax.partition_id_tensor())
        return tuple(bass2jax._bass_exec_p.bind(
            *operands, out_avals=tuple(out_avals), in_names=tuple(all_in),
            out_names=tuple(out_names), lowering_input_output_aliases=(),
            sim_require_finite=True, sim_require_nnan=True, nc=nc))

    devices = jax.devices()[:N_CORES]
    mesh = Mesh(np.asarray(devices), ("core",))
    jitted = jax.jit(
        shard_map(_body, mesh=mesh,
                  in_specs=(PartitionSpec("core"),) * (n_params + len(out_avals)),
                  out_specs=(PartitionSpec("core"),) * len(out_names),
                  check_rep=False),
        keep_unused=True)
    shard = NamedSharding(mesh, PartitionSpec("core"))
    state = (jitted, in_names, out_avals, mesh, shard)
    _CACHE["exec"] = state
    return state


def _prep_in_maps(essays, W_lstm, b_lstm, W_dense, b_dense):
    perm = np.concatenate([
        np.arange(1024, 1536),   # f
        np.arange(0, 512),       # i
        np.arange(512, 1024),    # j
        np.arange(1536, 2048),   # o
    ])
    Wx = W_lstm[:D][:, perm]
    Wh = W_lstm[D:][:, perm]
    b_eff = b_lstm[perm].astype(np.float32).copy()
    b_eff[0:512] += 1.0  # TF BasicLSTMCell forget bias ([f] block is first)

    Wx_pad = np.zeros((D_PAD, G4), np.float32)
    Wx_pad[:D] = Wx
    wx_packed = Wx_pad.reshape(3, 128, MC, 128).transpose(1, 0, 2, 3) \
        .reshape(128, 3 * MC * 128).astype(np.float16)
    import ml_dtypes
    wh_packed = (Wh * WS).reshape(KC, 128, MC, 128).transpose(1, 0, 2, 3) \
        .reshape(128, KC * MC * 128).astype(ml_dtypes.float8_e4m3)
    bias_t = b_eff.reshape(MC, 128).T.copy().astype(np.float32)
    wd_t = W_dense[:, 0].reshape(KC, 128).T.copy().astype(np.float32)
    bd_t = np.full((B, 1), float(b_dense[0]), np.float32)

    ex_pad = np.zeros((BF, T, D_PAD), np.float16)
    ex_pad[:, :, :D] = essays.astype(np.float16)

    return [{
        "ex": ex_pad[core * B:(core + 1) * B],
        "wh": wh_packed,
        "wx": wx_packed,
        "bias": bias_t,
        "wd": wd_t,
        "bd": bd_t,
        "ident": (np.eye(128, dtype=np.float32) * WS).astype(
            ml_dtypes.float8_e4m3),
    } for core in range(N_CORES)]


def kernel(essays, W_lstm, b_lstm, W_dense, b_dense):
    import jax
    essays = np.asarray(essays, np.float32)
    W_lstm = np.asarray(W_lstm, np.float32)
    b_lstm = np.asarray(b_lstm, np.float32)
    W_dense = np.asarray(W_dense, np.float32)
    b_dense = np.asarray(b_dense, np.float32)

    jitted, in_names, out_avals, mesh, shard = _get_exec()
    in_maps = _prep_in_maps(essays, W_lstm, b_lstm, W_dense, b_dense)
    concat_in = [np.concatenate([in_maps[c][nm] for c in range(N_CORES)],
                                axis=0) for nm in in_names]
    concat_zeros = [np.zeros((N_CORES * a.shape[0], *a.shape[1:]), a.dtype)
                    for a in out_avals]
    dev_in = [jax.device_put(a, shard) for a in concat_in]
    dev_zeros = [jax.device_put(a, shard) for a in concat_zeros]
    out = jitted(*dev_in, *dev_zeros)
    jax.block_until_ready(out)
    preds = np.asarray(out[0]).reshape(-1).astype(np.float32)
    return preds


# expose the device-resident runner for timing harnesses
def _timed_run(essays, W_lstm, b_lstm, W_dense, b_dense, n_launch=9,
               trials=4):
    """Return (preds, per_launch_seconds_median) using pipelined launches."""
    import time, jax
    jitted, in_names, out_avals, mesh, shard = _get_exec()
    in_maps = _prep_in_maps(np.asarray(essays, np.float32),
                            np.asarray(W_lstm, np.float32),
                            np.asarray(b_lstm, np.float32),
                            np.asarray(W_dense, np.float32),
                            np.asarray(b_dense, np.float32))
    concat_in = [np.concatenate([in_maps[c][nm] for c in range(N_CORES)],
                                axis=0) for nm in in_names]
    concat_zeros = [np.zeros((N_CORES * a.shape[0], *a.shape[1:]), a.dtype)
                    for a in out_avals]
    dev_in = [jax.device_put(a, shard) for a in concat_in]
    dev_zeros = [jax.device_put(a, shard) for a in concat_zeros]

    out = jitted(*dev_in, *dev_zeros)
    jax.block_until_ready(out)
    preds = np.asarray(out[0]).reshape(-1).astype(np.float32)

    def timed(K):
        t0 = time.perf_counter()
        o = None
        for _ in range(K):
            o = jitted(*dev_in, *dev_zeros)
        jax.block_until_ready(o)
        return time.perf_counter() - t0

    # pipelined-launch slope: marginal cost of 16 extra launches.  This is
    # an upper bound on device time (host dispatch overlaps device exec).
    timed(2)  # warm
    margins = []
    for _ in range(trials):
        t3 = timed(3)
        t19 = timed(19)
        margins.append((t19 - t3) / 16)
    return preds, float(np.median(margins))

